# revision 1
# baseline (speedup 1.0000x reference)
"""Trainium2 Bass kernel for nn_CodecAttention (sliding-window ALiBi attention).

Reference computation (B=4, T=2048, DIM=1024, H=8, HD=128, WINDOW=16):
    xq = rms_norm(x @ wq) ; xk = rms_norm(x @ wk) ; xv = x @ wv
    scores = q k^T / sqrt(HD) + alibi_bias  (causal + 16-token sliding window)
    out = softmax(scores) @ v  -> reshape -> @ wo

Sharding: 8 cores = (batch b, sequence half). Each core processes 1024 query
tokens plus a 128-token key/value halo (zeros for the first half), fully
locally -- the attention window (16) never crosses the halo, so no
collectives are needed.

Layout strategy (per core): everything transposed. Host passes xT [DIM, 1152].
Projections produce qT/kT in [dim, tok] layout and v in natural [tok, dim]
layout. Scores are computed transposed (sT[k, q] = kT.T @ qT per head), the
softmax denominator comes from a ones-column matmul (reduction over the
partition axis), and PV produces attn_outT[d, q] = v.T-free matmul with
exp(sT) as the moving operand. attn_outT is exactly the stationary operand the
wo matmul wants, so the final output lands in natural [tok, dim] layout with
zero transposes anywhere.

All matmuls run in float32r (full PE rate at N>=256, ~1.6e-4 rel err/K=128).
RMS norm: sum-of-squares via ones-matmul, rsqrt via ACT Sqrt + DVE reciprocal,
applied through a K=1 broadcast matmul (rstd per token broadcast across
partitions; the k-side broadcast uses q_norm_w*k_norm_w/sqrt(HD) as the
stationary operand, folding the norm weights and score scale in for free).

ALiBi + causal + window mask: tiny per-(j) rel tiles with -1e9 at invalid
positions; scores += slope_h * rel via one fused scalar_tensor_tensor. The
first key tile of the first q-chunk additionally subtracts a per-core
"negcol" column that kills out-of-range (global position < 0) halo keys.
"""

import math
import os

import numpy as np

os.environ.setdefault("MYCRO_LOCAL_CACHE", "1")

import concourse.mybir as mybir
import concourse.tile as tile
from concourse import bacc
from concourse.bass_utils import run_bass_kernel_spmd

F32 = mybir.dt.float32
F32R = mybir.dt.float32r
AF = mybir.ActivationFunctionType
ALU = mybir.AluOpType

B, T, DIM = 4, 2048, 1024
H, HD = 8, 128
WINDOW = 16
EPS = 1e-6
NEG = -1.0e9
BIGMASK = 30000.0

HALO = 128                 # key/value halo tokens per shard
TSH = HALO + T // 2        # 1152 tokens per shard
QTOK = T // 2              # 1024 query tokens per shard
ND = DIM // 128            # 8 dim tiles
NT = TSH // 128            # 9 token tiles
QC = 256                   # attention query-chunk width
NQC = QTOK // QC           # 4 query chunks
K_CHUNKS = [(0, 384), (384, 384), (768, 384)]        # kT projection chunks
Q_CHUNKS = [(0, 512), (512, 512)]                    # qT projection chunks

_SLOPES = [2.0 ** (-i) for i in range(H)]

_CACHE = {}


def _build_program():
    nc = bacc.Bacc("TRN2", debug=False, target_bir_lowering=False, num_devices=8)

    xt = nc.declare_dram_parameter("xt", [128, ND, TSH], F32R, isOutput=False)
    wq = nc.declare_dram_parameter("wq", [DIM, DIM], F32R, isOutput=False)
    wk = nc.declare_dram_parameter("wk", [DIM, DIM], F32R, isOutput=False)
    wv = nc.declare_dram_parameter("wv", [DIM, DIM], F32R, isOutput=False)
    wo = nc.declare_dram_parameter("wo", [DIM, DIM], F32R, isOutput=False)
    qkw_row = nc.declare_dram_parameter("qkw_row", [1, ND, 128], F32R, isOutput=False)
    ones_row = nc.declare_dram_parameter("ones_row", [1, 128], F32R, isOutput=False)
    ones_col = nc.declare_dram_parameter("ones_col", [128, 1], F32R, isOutput=False)
    rel4 = nc.declare_dram_parameter("rel4", [128, 4, QC], F32, isOutput=False)
    out = nc.declare_dram_parameter("out", [QTOK, DIM], F32, isOutput=True)

    with tile.TileContext(nc) as tc:
        with tc.tile_pool(name="big", bufs=1) as big:
            # ---- constants + persistent tensors (live for the whole kernel) ----
            kt_sb = big.tile([128, ND, TSH], F32R)
            qt_sb = big.tile([128, ND, QTOK], F32R)
            v_sb = big.tile([128, NT, DIM], F32R)
            qkw_sb = big.tile([1, ND, 128], F32R)
            onesr_sb = big.tile([1, 128], F32R)
            onesc_sb = big.tile([128, 1], F32R)
            rel4_sb = big.tile([128, 4, QC], F32)
            eps_sb = big.tile([1, 1], F32)
            nc.vector.memset(eps_sb[:], EPS)
            nc.sync.dma_start(qkw_sb[:], qkw_row[:])
            nc.sync.dma_start(onesr_sb[:], ones_row[:])
            nc.sync.dma_start(onesc_sb[:], ones_col[:])
            nc.sync.dma_start(rel4_sb[:], rel4[:])

            self_phase1(tc, nc, kt_sb, qt_sb, v_sb, qkw_sb, onesr_sb, onesc_sb,
                        eps_sb, xt, wq, wk, wv)
            self_phase2(tc, nc, kt_sb, qt_sb, v_sb, onesr_sb, onesc_sb,
                        rel4_sb, wo, out)
    nc.compile()
    return nc


def self_phase1(tc, nc, kt_sb, qt_sb, v_sb, qkw_sb, onesr_sb, onesc_sb,
                eps_sb, xt, wq, wk, wv):
    with (
        tc.tile_pool(name="xtp", bufs=1) as xtp,
        tc.tile_pool(name="wp", bufs=int(os.environ.get("KP_WP", 10))) as wp,
        tc.tile_pool(name="scr", bufs=2) as scrp,
        tc.tile_pool(name="sqt", bufs=1) as sqtp,
        tc.tile_pool(name="rcp", bufs=2) as rcpp,
        tc.tile_pool(name="pp", bufs=int(os.environ.get("KP_PP", 6)),
                     space="PSUM") as pp,
        tc.tile_pool(name="sqp", bufs=int(os.environ.get("KP_SQP", 1)),
                     space="PSUM") as sqp,
        tc.tile_pool(name="bcp", bufs=int(os.environ.get("KP_BCP", 1)),
                     space="PSUM") as bcp,
    ):
            xt_sb = xtp.tile([128, ND, TSH], F32R)

            # ---- projections: kT and qT (with RMS-norm), v (plain) ----
            def drain_ps(dst, ps, m, c0, cw, ssq):
                # raw copy (rounded to f32r) + square + ssq accumulation;
                # alternate engines to balance ACT vs DVE load
                if m % 2 == 0:
                    nc.scalar.copy(dst[:, m, c0:c0 + cw], ps[:, :cw])
                else:
                    nc.vector.tensor_copy(dst[:, m, c0:c0 + cw], ps[:, :cw])
                sq = scrp.tile([128, 512], F32R, tag="sq")
                if m % 2 == 0:
                    # DVE square must read the SBUF copy (one-PSUM-input rule)
                    nc.vector.tensor_mul(sq[:, :cw], dst[:, m, c0:c0 + cw],
                                         dst[:, m, c0:c0 + cw])
                else:
                    nc.scalar.square(sq[:, :cw], ps[:, :cw])
                nc.tensor.matmul(
                    ssq[:, :cw], onesc_sb[:], sq[:, :cw],
                    start=(m == 0), stop=(m == ND - 1),
                )

            def proj_normed(w_dram, dst, chunks, tok0, fold_qkw, first=False,
                            pool=None):
                pool = pool or pp
                """dst[:, m, c] = rstd * (x @ w)^T, rstd from raw sum-of-squares."""
                w_slices = []
                for kk in range(ND):
                    w_sl = wp.tile([128, DIM], F32R, tag="wslice")
                    nc.sync.dma_start(w_sl[:], w_dram[kk * 128:(kk + 1) * 128, :])
                    if first:
                        # interleave xt loads so the kk-outer first chunk can
                        # start as soon as the first (w, xt) slice pair lands
                        nc.sync.dma_start(xt_sb[:, kk, :], xt[:, kk, :])
                    w_slices.append(w_sl)
                for ci, (c0, cw) in enumerate(chunks):
                    ssq = sqp.tile([1, 512], F32)
                    if first and ci == 0:
                        # kk-outer in m-blocks of 4: PE consumes DMA'd slices
                        # incrementally instead of waiting for all 16
                        for mb in range(0, ND, 4):
                            blk = []
                            for m in range(mb, mb + 4):
                                ps = pool.tile([128, 512], F32, tag="ps")
                                blk.append(ps)
                            for kk in range(ND):
                                for mi, m in enumerate(range(mb, mb + 4)):
                                    nc.tensor.matmul(
                                        blk[mi][:, :cw],
                                        w_slices[kk][:, m * 128:(m + 1) * 128],
                                        xt_sb[:, kk, tok0 + c0: tok0 + c0 + cw],
                                        start=(kk == 0), stop=(kk == ND - 1),
                                    )
                            for mi, m in enumerate(range(mb, mb + 4)):
                                drain_ps(dst, blk[mi], m, c0, cw, ssq)
                    else:
                        for m in range(ND):
                            ps = pool.tile([128, 512], F32, tag="ps")
                            for kk in range(ND):
                                nc.tensor.matmul(
                                    ps[:, :cw],
                                    w_slices[kk][:, m * 128:(m + 1) * 128],
                                    xt_sb[:, kk, tok0 + c0: tok0 + c0 + cw],
                                    start=(kk == 0), stop=(kk == ND - 1),
                                )
                            drain_ps(dst, ps, m, c0, cw, ssq)
                    sqt = sqtp.tile([1, 512], F32, tag="sqt")
                    nc.scalar.activation(sqt[:, :cw], ssq[:, :cw], AF.Sqrt,
                                         bias=eps_sb[:], scale=1.0 / DIM)
                    rstd = rcpp.tile([1, 512], F32R, tag="rstd")
                    with nc.allow_low_precision(reason="f32r rstd for matmul"):
                        nc.vector.reciprocal(rstd[:, :cw], sqt[:, :cw])
                    if fold_qkw:
                        for m in range(ND):
                            rsb = bcp.tile([128, 512], F32)
                            nc.tensor.matmul(rsb[:, :cw], qkw_sb[:, m, :],
                                             rstd[:, :cw], start=True, stop=True)
                            nc.vector.tensor_mul(dst[:, m, c0:c0 + cw],
                                                 dst[:, m, c0:c0 + cw], rsb[:, :cw])
                    else:
                        rsb = bcp.tile([128, 512], F32)
                        nc.tensor.matmul(rsb[:, :cw], onesr_sb[:],
                                         rstd[:, :cw], start=True, stop=True)
                        # stage the broadcast in SBUF: frees the psum slot and
                        # keeps the 8 muls off the one-PSUM-operand path
                        rsb_sb = scrp.tile([128, 512], F32, tag="rsbsb")
                        nc.scalar.copy(rsb_sb[:, :cw], rsb[:, :cw])
                        for m in range(ND):
                            nc.vector.tensor_mul(dst[:, m, c0:c0 + cw],
                                                 dst[:, m, c0:c0 + cw],
                                                 rsb_sb[:, :cw])

            proj_normed(wk, kt_sb, K_CHUNKS, 0, fold_qkw=True, first=True)
            proj_normed(wq, qt_sb, Q_CHUNKS, HALO, fold_qkw=False)

            # v: natural layout [tok, dim]
            wv_slices = []
            for kk in range(ND):
                w_sl = wp.tile([128, DIM], F32R, tag="wslice")
                nc.sync.dma_start(w_sl[:], wv[kk * 128:(kk + 1) * 128, :])
                wv_slices.append(w_sl)
            for tt in range(NT):
                for nn in range(2):
                    ps = pp.tile([128, 512], F32)
                    for kk in range(ND):
                        nc.tensor.matmul(
                            ps[:],
                            xt_sb[:, kk, tt * 128:(tt + 1) * 128],
                            wv_slices[kk][:, nn * 512:(nn + 1) * 512],
                            start=(kk == 0), stop=(kk == ND - 1),
                        )
                    if tt % 2 == 0:
                        nc.scalar.copy(v_sb[:, tt, nn * 512:(nn + 1) * 512], ps[:])
                    else:
                        nc.vector.tensor_copy(v_sb[:, tt, nn * 512:(nn + 1) * 512],
                                              ps[:])


def self_phase2(tc, nc, kt_sb, qt_sb, v_sb, onesr_sb, onesc_sb,
                rel4_sb, wo, out):
        # xt freed; load wo and run attention + output projection
        with (
            tc.tile_pool(name="wo", bufs=1) as wop,
            tc.tile_pool(name="exp", bufs=int(os.environ.get("KP_EXP", 3))) as expp,
            tc.tile_pool(name="atc", bufs=int(os.environ.get("KP_ATC", 2))) as atcp,
            tc.tile_pool(name="outp", bufs=3) as outp,
            tc.tile_pool(name="rcp2", bufs=2) as rcp2p,
            tc.tile_pool(name="sps", bufs=int(os.environ.get("KP_SPS", 2)),
                         space="PSUM") as sps,
            tc.tile_pool(name="ytp", bufs=int(os.environ.get("KP_YTP", 1)),
                         space="PSUM") as ytp,
            tc.tile_pool(name="rsp", bufs=int(os.environ.get("KP_RSP", 1)),
                         space="PSUM") as rsp,
            tc.tile_pool(name="bc2", bufs=int(os.environ.get("KP_BC2", 1)),
                         space="PSUM") as bc2p,
            tc.tile_pool(name="pso", bufs=int(os.environ.get("KP_PSO", 1)),
                         space="PSUM") as psop,
        ):
            wo_sb = wop.tile([128, ND, DIM], F32R)
            for hd in range(ND):
                nc.sync.dma_start(wo_sb[:, hd, :], wo[hd * 128:(hd + 1) * 128, :])

            for qc in range(NQC):
                aT = atcp.tile([128, ND, QC], F32R)
                for h in range(H):
                    yT_t = ytp.tile([128, QC], F32, tag="yT")
                    rs_t = rsp.tile([1, QC], F32, tag="rs")
                    yT = yT_t[:, :]
                    rs = rs_t[:, :]
                    # joint [128, 3, QC] score tile: three QK matmuls, then ONE
                    # fused bias-add and ONE exp over all 768 columns.
                    # rel4 slots: [0]=j0-first-tile variant (per-core: all-NEG
                    # on first-half cores), [1]=j1, [2]=j2, [3]=j0-regular.
                    # qc=0 uses rel4[0:3] with slots (j0,j1,j2); qc>0 uses
                    # rel4[1:4] with slots (j1,j2,j0).
                    jmap = (0, 1, 2) if qc == 0 else (1, 2, 0)
                    rel_w = rel4_sb[:, 0:3, :] if qc == 0 else rel4_sb[:, 1:4, :]
                    stj = sps.tile([128, 3, QC], F32)
                    st = stj[:, 0:3, :]
                    for s, j in enumerate(jmap):
                        nc.tensor.matmul(
                            stj[:, s, :],
                            kt_sb[:, h, qc * QC + j * 128: qc * QC + (j + 1) * 128],
                            qt_sb[:, h, qc * QC: (qc + 1) * QC],
                            start=True, stop=True,
                        )
                    # scores += slope_h * rel (rel = -1e9 at masked positions)
                    nc.vector.scalar_tensor_tensor(
                        out=st[:], in0=rel_w, scalar=_SLOPES[h],
                        in1=st[:], op0=ALU.mult, op1=ALU.add)
                    ex = expp.tile([128, 3, QC], F32R, tag="exp")
                    nc.scalar.activation(ex[:], st[:], AF.Exp)
                    for s, j in enumerate(jmap):
                        nc.tensor.matmul(
                            yT,
                            v_sb[:, 2 * qc + j, h * 128:(h + 1) * 128],
                            ex[:, s, :], start=(s == 0), stop=(s == 2),
                        )
                        nc.tensor.matmul(
                            rs, onesc_sb[:], ex[:, s, :],
                            start=(s == 0), stop=(s == 2),
                        )
                    rcp = rcp2p.tile([1, QC], F32R, tag="rcp")
                    with nc.allow_low_precision(reason="f32r prob scale"):
                        nc.vector.reciprocal(rcp[:], rs)
                    rsb2_t = bc2p.tile([128, QC], F32, tag="rsb2")
                    rsb2 = rsb2_t[:, :]
                    nc.tensor.matmul(rsb2, onesr_sb[:], rcp[:],
                                     start=True, stop=True)
                    nc.scalar.copy(aT[:, h, :], yT)
                    nc.vector.tensor_mul(aT[:, h, :], aT[:, h, :], rsb2)

                # output projection for this q-chunk
                for t2 in range(QC // 128):
                    for nn in range(2):
                        ps_o = psop.tile([128, 512], F32)
                        for hd in range(ND):
                            nc.tensor.matmul(
                                ps_o[:],
                                aT[:, hd, t2 * 128:(t2 + 1) * 128],
                                wo_sb[:, hd, nn * 512:(nn + 1) * 512],
                                start=(hd == 0), stop=(hd == ND - 1),
                            )
                        o_sb = outp.tile([128, 512], F32, tag="osb")
                        nc.vector.tensor_copy(o_sb[:], ps_o[:])
                        nc.sync.dma_start(
                            out[qc * QC + t2 * 128: qc * QC + (t2 + 1) * 128,
                                nn * 512:(nn + 1) * 512],
                            o_sb[:],
                        )


def _host_constants():
    # relpat(j)[kj, qi] = 128*(j-1) + kj - qi if in window else NEG
    kj = np.arange(128)[:, None, None]
    jj = np.arange(3)[None, :, None]
    qi = np.arange(QC)[None, None, :]
    rel = 128 * (jj - 1) + kj - qi
    valid = (rel <= 0) & (rel >= -WINDOW)
    relpat = np.where(valid, rel, NEG).astype(np.float32)  # [128, 3, QC]
    ones_row = np.ones((1, 128), dtype=np.float32)
    ones_col = np.ones((128, 1), dtype=np.float32)
    return relpat, ones_row, ones_col


def _make_in_maps(x, wq, wk, wv, wo, q_norm_w, k_norm_w):
    x = np.ascontiguousarray(np.asarray(x, dtype=np.float32))
    wq = np.ascontiguousarray(np.asarray(wq, dtype=np.float32))
    wk = np.ascontiguousarray(np.asarray(wk, dtype=np.float32))
    wv = np.ascontiguousarray(np.asarray(wv, dtype=np.float32))
    wo = np.ascontiguousarray(np.asarray(wo, dtype=np.float32))
    q_norm_w = np.asarray(q_norm_w, dtype=np.float32)
    k_norm_w = np.asarray(k_norm_w, dtype=np.float32)

    relpat, ones_row, ones_col = _host_constants()
    qkw = (q_norm_w * k_norm_w / math.sqrt(HD)).astype(np.float32)
    qkw_row = qkw.reshape(1, ND, 128)

    in_maps = []
    for c in range(8):
        b, hf = c // 2, c % 2
        base = hf * (T // 2)
        xsh = np.zeros((TSH, DIM), dtype=np.float32)
        lo = base - HALO
        if lo < 0:
            xsh[HALO:] = x[b, base: base + QTOK]
        else:
            xsh[:] = x[b, lo: base + QTOK]
        xt_c = np.ascontiguousarray(
            xsh.T.reshape(ND, 128, TSH).transpose(1, 0, 2))
        rel4 = np.empty((128, 4, QC), dtype=np.float32)
        rel4[:, 1:3, :] = relpat[:, 1:3, :]          # j1, j2
        rel4[:, 3, :] = relpat[:, 0, :]              # j0 regular
        rel4[:, 0, :] = NEG if hf == 0 else relpat[:, 0, :]  # j0 first tile
        in_maps.append({
            "xt": xt_c, "wq": wq, "wk": wk, "wv": wv, "wo": wo,
            "qkw_row": qkw_row, "ones_row": ones_row, "ones_col": ones_col,
            "rel4": rel4,
        })

    return in_maps


def kernel(x, wq, wk, wv, wo, q_norm_w, k_norm_w):
    if "nc" not in _CACHE:
        _CACHE["nc"] = _build_program()
    nc = _CACHE["nc"]
    in_maps = _make_in_maps(x, wq, wk, wv, wo, q_norm_w, k_norm_w)
    _CACHE["in_maps"] = in_maps
    import time as _time
    last_err = None
    for attempt in range(3):
        try:
            res = run_bass_kernel_spmd(nc, in_maps, core_ids=list(range(8)))
            break
        except Exception as e:  # transient NRT/device wedges recover on retry
            last_err = e
            _time.sleep(10 * (attempt + 1))
    else:
        raise last_err

    out = np.empty((B, T, DIM), dtype=np.float32)
    for c in range(8):
        b, hf = c // 2, c % 2
        out[b, hf * QTOK:(hf + 1) * QTOK, :] = res.results[c]["out"]
    return out



# revision 9
# speedup vs baseline: 1.2337x; 1.2337x over previous
"""Trainium2 Bass kernel for nn_CodecAttention (sliding-window ALiBi attention).

Reference computation (B=4, T=2048, DIM=1024, H=8, HD=128, WINDOW=16):
    xq = rms_norm(x @ wq) ; xk = rms_norm(x @ wk) ; xv = x @ wv
    scores = q k^T / sqrt(HD) + alibi_bias  (causal + 16-token sliding window)
    out = softmax(scores) @ v  -> reshape -> @ wo

Sharding: 8 cores = (batch b, sequence half). Each core processes 1024 query
tokens plus a 16-token key/value halo (zeros for the first half), fully
locally -- no collectives.

All tensors bf16 (PSUM accumulation f32); the test-relative error budget
(2e-2) dwarfs bf16 noise (~5e-3 end to end). Matmul cost in the perf model is
moving-width * cycle regardless of dtype/K, so the kernel minimizes total
moving rows:

  - projections kT/qT in [feature, token] layout (cost ~ tokens), v in
    [token, feature] layout (needed as PV stationary).
  - RMS norm: squares via one DVE scalar_tensor_tensor per m-tile (folding the
    norm-weight correction), partition-reduction via a single ones[128,128]
    matmul per chunk that also broadcasts the result to all partitions, then
    one fused ACT Rsqrt -> per-token multiply. q_norm_w*k_norm_w/sqrt(HD) is
    folded into wk host-side (inverse-square correction via the stt scalar).
  - attention per (head, 128-query tile): scores in [query, key] layout via a
    single 144-wide QK matmul (16-wide band halo); exp on ACT; mask+ALiBi via
    a host-precomputed exp(bias) pattern multiplied in on DVE (masked
    positions are exactly 0) which also emits the softmax denominator via
    accum_out; probs normalized per-partition (tensor_scalar with [128,1]
    reciprocal); transposed via PE back to [key, query] for PV.
  - output projection from the [head-dim, token] attention output, landing in
    natural [token, dim] layout for the store.
"""

import math
import os

import numpy as np
import ml_dtypes

os.environ.setdefault("MYCRO_LOCAL_CACHE", "1")

import concourse.mybir as mybir
import concourse.tile as tile
from concourse import bacc
from concourse.bass_utils import run_bass_kernel_spmd

F32 = mybir.dt.float32
BF16 = mybir.dt.bfloat16
AF = mybir.ActivationFunctionType
ALU = mybir.AluOpType

B, T, DIM = 4, 2048, 1024
H, HD = 8, 128
WINDOW = 16
EPS = 1e-6

HALO = 16                  # key/value halo tokens per shard
TSH = HALO + T // 2        # 1040 k/v tokens per shard
QTOK = T // 2              # 1024 query tokens per shard
ND = DIM // 128            # 8 dim tiles
NVT = 9                    # v token tiles (8*128 + 16)
NQT = QTOK // 128          # 8 query tiles
KW = 128 + HALO            # 144 keys per query tile

_SLOPES = [2.0 ** (-i) for i in range(H)]

_CACHE = {}


def _build_program():
    nc = bacc.Bacc("TRN2", debug=False, target_bir_lowering=False, num_devices=8)

    xt = nc.declare_dram_parameter("xt", [128, ND, TSH], BF16, isOutput=False)
    wq = nc.declare_dram_parameter("wq", [DIM, DIM], BF16, isOutput=False)
    wk = nc.declare_dram_parameter("wk", [DIM, DIM], BF16, isOutput=False)
    wv = nc.declare_dram_parameter("wv", [DIM, DIM], BF16, isOutput=False)
    wo = nc.declare_dram_parameter("wo", [DIM, DIM], BF16, isOutput=False)
    invu2 = nc.declare_dram_parameter("invu2", [128, ND], F32, isOutput=False)
    ident = nc.declare_dram_parameter("ident", [128, 128], BF16, isOutput=False)
    expb = nc.declare_dram_parameter("expb", [128, H, 2, KW], BF16, isOutput=False)
    out = nc.declare_dram_parameter("out", [QTOK, DIM], F32, isOutput=True)

    with tile.TileContext(nc) as tc:
        with tc.tile_pool(name="big", bufs=1) as big:
            kt_sb = big.tile([128, ND, TSH], BF16)
            qt_sb = big.tile([128, ND, QTOK], BF16)
            v_sb = big.tile([128, NVT, DIM], BF16)
            wo_sb = big.tile([128, ND, DIM], BF16)
            invu2_sb = big.tile([128, ND], F32)
            ident_sb = big.tile([128, 128], BF16)
            expb_sb = big.tile([128, H, 2, KW], BF16)
            ones_sb = big.tile([128, 128], BF16)
            eps_sb = big.tile([128, 1], F32)
            nc.vector.memset(eps_sb[:], EPS)
            nc.vector.memset(ones_sb[:], 1.0)

            self_phase1(tc, nc, kt_sb, qt_sb, v_sb, wo_sb, invu2_sb, ident_sb,
                        expb_sb, eps_sb, ones_sb, xt, wq, wk, wv, wo, invu2,
                        ident, expb)
            self_phase2(tc, nc, kt_sb, qt_sb, v_sb, wo_sb, ident_sb, expb_sb,
                        ones_sb, out)
    nc.compile()
    return nc


def self_phase1(tc, nc, kt_sb, qt_sb, v_sb, wo_sb, invu2_sb, ident_sb,
                expb_sb, eps_sb, ones_sb, xt, wq, wk, wv, wo, invu2, ident,
                expb):
    with (
        tc.tile_pool(name="xtp", bufs=1) as xtp,
        tc.tile_pool(name="wp", bufs=int(os.environ.get("KP_WP", 18))) as wp,
        tc.tile_pool(name="sq", bufs=int(os.environ.get("KP_SQ", 18))) as sqp,
        tc.tile_pool(name="rst", bufs=2) as rstp,
        tc.tile_pool(name="pp2", bufs=int(os.environ.get("KP_PP2", 3)),
                     space="PSUM") as pp2,
        tc.tile_pool(name="pps", bufs=int(os.environ.get("KP_PPS", 2)),
                     space="PSUM") as pps,
    ):
        xt_sb = xtp.tile([128, ND, TSH], BF16)

        wk_sl = []
        for kk in range(ND):
            w_sl = wp.tile([128, DIM], BF16, tag="wslice")
            nc.sync.dma_start(w_sl[:], wk[kk * 128:(kk + 1) * 128, :])
            nc.sync.dma_start(xt_sb[:, kk, :], xt[:, kk, :])
            wk_sl.append(w_sl)
        # small constants after the first couple of slice pairs are queued
        nc.sync.dma_start(invu2_sb[:], invu2[:])
        nc.sync.dma_start(ident_sb[:], ident[:])

        def drain_sq(dst, ps, m, c0, cw, scal, sq_tiles, par):
            """PSUM -> bf16 SBUF copy + raw-square (weighted) for the rms."""
            if par % 2 == 0:
                nc.scalar.copy(dst[:, m, c0:c0 + cw], ps[:, :cw])
            else:
                nc.vector.tensor_copy(dst[:, m, c0:c0 + cw], ps[:, :cw])
            sq = sqp.tile([128, 512], BF16, tag="sqt")
            nc.vector.scalar_tensor_tensor(
                out=sq[:, :cw], in0=dst[:, m, c0:c0 + cw], scalar=scal,
                in1=dst[:, m, c0:c0 + cw], op0=ALU.mult, op1=ALU.mult)
            sq_tiles.append(sq)

        def rms_apply(dst, sq_tiles, c0, cw):
            """Pool-tree sum of 8 sq tiles -> ones-matmul (partition reduce +
            broadcast) -> Rsqrt -> 8 per-token multiplies."""
            lvl = sq_tiles
            eng = [nc.gpsimd, nc.vector]
            li = 0
            while len(lvl) > 1:
                nxt = []
                for i in range(0, len(lvl) - 1, 2):
                    acc = sqp.tile([128, 512], BF16, tag="sqacc")
                    eng[li % 2].tensor_add(acc[:, :cw], lvl[i][:, :cw],
                                           lvl[i + 1][:, :cw])
                    nxt.append(acc)
                if len(lvl) % 2:
                    nxt.append(lvl[-1])
                lvl = nxt
                li += 1
            ssqb = pps.tile([128, 512], F32, tag="ps")
            nc.tensor.matmul(ssqb[:, :cw], ones_sb[:], lvl[0][:, :cw],
                             start=True, stop=True)
            sqt = rstp.tile([128, 512], F32, tag="sqt")
            nc.scalar.activation(sqt[:, :cw], ssqb[:, :cw], AF.Sqrt,
                                 bias=eps_sb[:], scale=1.0)
            rstd = rstp.tile([128, 512], BF16, tag="rstd")
            with nc.allow_low_precision(reason="bf16 rstd"):
                nc.vector.reciprocal(rstd[:, :cw], sqt[:, :cw])
            for m in range(ND):
                nc.vector.tensor_mul(dst[:, m, c0:c0 + cw],
                                     dst[:, m, c0:c0 + cw], rstd[:, :cw])

        # ---- k projection: chunk-pair pass (kk-outer, m-blocks of 3) ----
        sqk = [[], []]
        for mb in range(0, ND, 3):
            ms = list(range(mb, min(mb + 3, ND)))
            blk = {}
            for m in ms:
                blk[m] = pp2.tile([128, 2, 512], F32, tag="ps2", name="ps2")
            for kk in range(ND):
                for m in ms:
                    for ci in range(2):
                        nc.tensor.matmul(
                            blk[m][:, ci, :],
                            wk_sl[kk][:, m * 128:(m + 1) * 128],
                            xt_sb[:, kk, ci * 512:(ci + 1) * 512],
                            start=(kk == 0), stop=(kk == ND - 1))
            for m in ms:
                for ci in range(2):
                    drain_sq(kt_sb, blk[m][:, ci, :], m, ci * 512, 512,
                             invu2_sb[:, m:m + 1], sqk[ci], m + ci)
        # 16-wide k tail chunk
        ps16 = pps.tile([128, 512], F32, tag="ps")
        sqk2 = []
        for m in range(ND):
            for kk in range(ND):
                nc.tensor.matmul(ps16[:, m * 16:(m + 1) * 16],
                                 wk_sl[kk][:, m * 128:(m + 1) * 128],
                                 xt_sb[:, kk, 1024:1040],
                                 start=(kk == 0), stop=(kk == ND - 1))
            drain_sq(kt_sb, ps16[:, m * 16:(m + 1) * 16], m, 1024, 16,
                     invu2_sb[:, m:m + 1], sqk2, m)
        rms_apply(kt_sb, sqk[0], 0, 512)
        rms_apply(kt_sb, sqk[1], 512, 512)
        rms_apply(kt_sb, sqk2, 1024, 16)

        # ---- q projection (m-outer; slices stream in during k pass) ----
        wq_sl = []
        for kk in range(ND):
            w_sl = wp.tile([128, DIM], BF16, tag="wslice")
            nc.sync.dma_start(w_sl[:], wq[kk * 128:(kk + 1) * 128, :])
            wq_sl.append(w_sl)
        wv_sl = []
        for kk in range(ND):
            w_sl = wp.tile([128, DIM], BF16, tag="wslice")
            nc.sync.dma_start(w_sl[:], wv[kk * 128:(kk + 1) * 128, :])
            wv_sl.append(w_sl)
        for hd in range(ND):
            nc.sync.dma_start(wo_sb[:, hd, :], wo[hd * 128:(hd + 1) * 128, :])
        nc.sync.dma_start(expb_sb[:], expb[:])

        for ci in range(2):
            c0 = HALO + ci * 512
            sqq = []
            for mp in range(0, ND, 2):
                ps = pp2.tile([128, 2, 512], F32, tag="ps2", name="ps2")
                for mi in range(2):
                    for kk in range(ND):
                        nc.tensor.matmul(ps[:, mi, :],
                                         wq_sl[kk][:, (mp + mi) * 128:(mp + mi + 1) * 128],
                                         xt_sb[:, kk, c0:c0 + 512],
                                         start=(kk == 0), stop=(kk == ND - 1))
                for mi in range(2):
                    drain_sq(qt_sb, ps[:, mi, :], mp + mi, ci * 512, 512,
                             1.0 / DIM, sqq, mp + mi)
            rms_apply(qt_sb, sqq, ci * 512, 512)

        # ---- v projection: natural [token, feature] layout ----
        for tt in range(NVT):
            tw = 128 if tt < 8 else 16
            ps = pp2.tile([128, 2, 512], F32, tag="ps2", name="ps2")
            for nn in range(2):
                for kk in range(ND):
                    nc.tensor.matmul(
                        ps[:tw, nn, :],
                        xt_sb[:, kk, tt * 128:tt * 128 + tw],
                        wv_sl[kk][:, nn * 512:(nn + 1) * 512],
                        start=(kk == 0), stop=(kk == ND - 1))
            for nn in range(2):
                if (tt + nn) % 2 == 0:
                    nc.scalar.copy(v_sb[:tw, tt, nn * 512:(nn + 1) * 512],
                                   ps[:tw, nn, :])
                else:
                    nc.vector.tensor_copy(
                        v_sb[:tw, tt, nn * 512:(nn + 1) * 512], ps[:tw, nn, :])


def self_phase2(tc, nc, kt_sb, qt_sb, v_sb, wo_sb, ident_sb, expb_sb,
                ones_sb, out):
    with (
        tc.tile_pool(name="ex", bufs=int(os.environ.get("KP_EX", 4))) as exp_,
        tc.tile_pool(name="exn", bufs=int(os.environ.get("KP_EXN", 4))) as exnp,
        tc.tile_pool(name="ext", bufs=int(os.environ.get("KP_EXT", 3))) as extp,
        tc.tile_pool(name="rs", bufs=4) as rsp,
        tc.tile_pool(name="at", bufs=int(os.environ.get("KP_AT", 2))) as atp,
        tc.tile_pool(name="ob", bufs=3) as obp,
        tc.tile_pool(name="sq2", bufs=int(os.environ.get("KP_SQ2", 2)),
                     space="PSUM") as sqp2,
        tc.tile_pool(name="tp", bufs=int(os.environ.get("KP_TP", 2)),
                     space="PSUM") as tpp,
        tc.tile_pool(name="yt", bufs=int(os.environ.get("KP_YT", 2)),
                     space="PSUM") as ytp,
        tc.tile_pool(name="po", bufs=int(os.environ.get("KP_PO", 2)),
                     space="PSUM") as pop,
    ):
        for t in range(NQT):
            aT = atp.tile([128, ND, 128], BF16)
            var = 0 if t == 0 else 1
            for h in range(H):
                sQ = sqp2.tile([128, KW], F32, tag="sq")
                nc.tensor.matmul(sQ[:],
                                 qt_sb[:, h, t * 128:(t + 1) * 128],
                                 kt_sb[:, h, t * 128:t * 128 + KW],
                                 start=True, stop=True)
                ex = exp_.tile([128, KW], BF16, tag="ex")
                nc.scalar.activation(ex[:], sQ[:], AF.Exp)
                exm = exnp.tile([128, KW], BF16, tag="exm")
                rs = rsp.tile([128, 1], F32, tag="rs")
                nc.vector.scalar_tensor_tensor(
                    out=exm[:], in0=ex[:], scalar=1.0,
                    in1=expb_sb[:, h, var, :], op0=ALU.mult, op1=ALU.mult,
                    accum_out=rs[:])
                rcp = rsp.tile([128, 1], F32, tag="rcp")
                nc.vector.reciprocal(rcp[:], rs[:])
                exn = exnp.tile([128, KW], BF16, tag="exn")
                nc.vector.tensor_scalar_mul(exn[:], exm[:], rcp[:])
                tp = tpp.tile([128, 2, 128], BF16, tag="tp")
                nc.tensor.transpose(tp[:, 0, :], exn[:, 0:128], ident_sb[:])
                nc.tensor.transpose(tp[0:16, 1, :], exn[:, 128:KW],
                                    ident_sb[:])
                exsT = extp.tile([128, 2, 128], BF16, tag="exsT")
                nc.scalar.copy(exsT[:], tp[:])
                yT = ytp.tile([128, 128], F32, tag="yT")
                hs = slice(h * 128, (h + 1) * 128)
                nc.tensor.matmul(yT[:], v_sb[:, t, hs], exsT[:, 0, :],
                                 start=True, stop=False)
                nc.tensor.matmul(yT[:], v_sb[0:16, t + 1, hs],
                                 exsT[0:16, 1, :], start=False, stop=True)
                if h % 2 == 0:
                    nc.vector.tensor_copy(aT[:, h, :], yT[:])
                else:
                    nc.scalar.copy(aT[:, h, :], yT[:])

            for nn in range(2):
                ps_o = pop.tile([128, 512], F32)
                for hd in range(ND):
                    nc.tensor.matmul(ps_o[:],
                                     aT[:, hd, :],
                                     wo_sb[:, hd, nn * 512:(nn + 1) * 512],
                                     start=(hd == 0), stop=(hd == ND - 1))
                o_sb = obp.tile([128, 512], F32, tag="osb")
                if nn % 2 == 0:
                    nc.vector.tensor_copy(o_sb[:], ps_o[:])
                else:
                    nc.scalar.copy(o_sb[:], ps_o[:])
                nc.sync.dma_start(
                    out[t * 128:(t + 1) * 128, nn * 512:(nn + 1) * 512],
                    o_sb[:])


def _host_constants():
    # expb[i, c] = exp(slope * (c - i - 16)) inside the band (0 <= c-i <= 16),
    # else 0.  Variant 0 masks kt cols < 16 (halo before sequence start).
    ii = np.arange(128)[:, None]
    cc = np.arange(KW)[None, :]
    rel = cc - ii - HALO
    band = (rel <= 0) & (rel >= -WINDOW)
    expb = np.zeros((128, H, 2, KW), dtype=np.float32)
    for h in range(H):
        pat = np.where(band, np.exp(_SLOPES[h] * rel), 0.0)
        expb[:, h, 1, :] = pat
        expb[:, h, 0, :] = np.where(cc < HALO, 0.0, pat)
    ident = np.eye(128, dtype=np.float32)
    return expb, ident


def _make_in_maps(x, wq, wk, wv, wo, q_norm_w, k_norm_w):
    bf16 = ml_dtypes.bfloat16
    x = np.asarray(x, dtype=np.float32)
    wq = np.asarray(wq, dtype=np.float32)
    wk = np.asarray(wk, dtype=np.float32)
    wv = np.asarray(wv, dtype=np.float32)
    wo = np.asarray(wo, dtype=np.float32)
    q_norm_w = np.asarray(q_norm_w, dtype=np.float32)
    k_norm_w = np.asarray(k_norm_w, dtype=np.float32)

    u = (q_norm_w * k_norm_w / math.sqrt(HD)).astype(np.float32)
    wk_f = (wk * u[None, :]).astype(bf16)
    # raw sum-of-squares correction: sum_f k_raw^2 = sum_f (k'_f)^2 / u_f^2
    invu2 = np.ascontiguousarray(
        (1.0 / (u * u * DIM)).reshape(ND, 128).T.astype(np.float32))

    expb, ident = _host_constants()
    wq_b = wq.astype(bf16)
    wv_b = wv.astype(bf16)
    wo_b = wo.astype(bf16)
    ident_b = ident.astype(bf16)

    in_maps = []
    for c in range(8):
        b, hf = c // 2, c % 2
        base = hf * (T // 2)
        xsh = np.zeros((TSH, DIM), dtype=np.float32)
        lo = base - HALO
        if lo < 0:
            xsh[HALO:] = x[b, base: base + QTOK]
        else:
            xsh[:] = x[b, lo: base + QTOK]
        xt_c = np.ascontiguousarray(
            xsh.T.reshape(ND, 128, TSH).transpose(1, 0, 2)).astype(bf16)
        expb_c = expb.copy()
        if hf == 1:
            expb_c[:, :, 0, :] = expb_c[:, :, 1, :]
        in_maps.append({
            "xt": xt_c, "wq": wq_b, "wk": wk_f, "wv": wv_b, "wo": wo_b,
            "invu2": invu2, "ident": ident_b,
            "expb": np.ascontiguousarray(expb_c.astype(bf16)),
        })

    return in_maps


def kernel(x, wq, wk, wv, wo, q_norm_w, k_norm_w):
    if "nc" not in _CACHE:
        _CACHE["nc"] = _build_program()
    nc = _CACHE["nc"]
    in_maps = _make_in_maps(x, wq, wk, wv, wo, q_norm_w, k_norm_w)
    _CACHE["in_maps"] = in_maps
    import time as _time
    last_err = None
    for attempt in range(3):
        try:
            res = run_bass_kernel_spmd(nc, in_maps, core_ids=list(range(8)))
            break
        except Exception as e:  # transient NRT/device wedges recover on retry
            last_err = e
            _time.sleep(10 * (attempt + 1))
    else:
        raise last_err

    out = np.empty((B, T, DIM), dtype=np.float32)
    for c in range(8):
        b, hf = c // 2, c % 2
        out[b, hf * QTOK:(hf + 1) * QTOK, :] = res.results[c]["out"]
    return out
